# revision 1
# baseline (speedup 1.0000x reference)
"""Trainium2 Bass kernel for nn_CustomLoss_50843822850472.

Computes, for L2-normalized rows f of `features` [8192, 128]:
    sim = f @ f.T                      (diagonal excluded)
    e   = exp(sim / TAU)
    P_i = sum_j e_ij over {sim_ij >= alpha, j != i}   (positive mass)
    S_i = sum_j e_ij over {j != i}                    (total mass)
    loss = mean_i [ log(S_i + 2eps) - log(P_i + eps) ]   (== reference)

Sharding: rows are split across 8 NeuronCores (1024 rows/core). Each core
receives the full normalized feature matrix, pre-transposed to [D=128, N=8192]
(fp16) and COLUMN-ROTATED by its row offset, so that every core's diagonal
block lands at local columns [m*128, m*128+128) of row-block m — making the
program identical (SPMD) across cores with static [128,128] diagonal
constants.

Per core, per (row-block m in 0..7, column-chunk k in 0..3):
  - 4x matmul (fp16 in, fp32 accum, K=128, M=128, N=512) -> PSUM [128, 2048]
  - (k==0) an extra accumulate-matmul adds -60000*I on the diagonal 128
    columns (start=False), driving those sims to -6e4 so exp -> 0
  - ACT computes E' = exp((sim - C)/TAU) -> fp16 SBUF with fused row-sum
    accum -> S' partial (fp32).  C ~= alpha keeps E' in fp16 range and puts
    the positive threshold at exp((alpha-C)/TAU) (= 1.0 when C == alpha).
  - DVE in-place E' = (E' >= beta') * E' with fused row-sum -> P' partial.
Host rescales by exp(C/TAU) (in float64) and assembles the mean loss.

Partial accumulators (P per half-block, S per chunk) are DMA'd out raw and
combined on the host.
"""
import sys

sys.path.insert(0, "/opt/trn_rl_repo")

import numpy as np

TAU = 0.07
EPS = 1e-10
DIAG_NEG = -60000.0     # fp16-exact; sim + DIAG_NEG -> exp underflows to 0

N = 8192
D = 128
NCORES = 8
R = N // NCORES          # rows per core
NBLK = R // 128          # row blocks per core
CHUNK = 2048             # columns per PSUM chunk (4 banks)
NCHUNK = N // CHUNK
_CACHE = {}
LAST_RESULT = None
PROFILE = False


def _piece_plan():
    """stt column pieces per row-block: fine-grained on the first block
    (earlier DVE start) and last block (shorter exposed tail)."""
    plan = []
    for m in range(NBLK):
        if m == 0 or m == NBLK - 1:
            plan.append([(k * CHUNK, (k + 1) * CHUNK) for k in range(NCHUNK)])
        else:
            plan.append([(0, N // 2), (N // 2, N)])
    return plan


def _shift_center(alpha: float) -> float:
    # E' = exp((sim - C)/TAU) must fit fp16: sim <= ~1.0002, so C >= ~0.23
    # keeps max E' < 65504/some margin. C == alpha puts the threshold at 1.0.
    return float(min(max(alpha, 0.30), 1.0))


def _build(alpha: float):
    import concourse.mybir as mybir
    from concourse import bacc, tile

    f32 = mybir.dt.float32
    f16 = mybir.dt.float16
    Alu = mybir.AluOpType

    c = _shift_center(alpha)
    betap = float(np.exp((np.float64(alpha) - c) / TAU))
    bias = float(-c / TAU)

    nc = bacc.Bacc(
        "TRN2", target_bir_lowering=False, debug=False, num_devices=NCORES
    )
    ft_d = nc.dram_tensor("ft", [128, N], f16, kind="ExternalInput")
    ident_d = nc.dram_tensor("ident", [128, 128], f16, kind="ExternalInput")
    negd_d = nc.dram_tensor("negd", [128, 128], f16, kind="ExternalInput")
    npcols = sum(len(p) for p in _piece_plan())
    out_d = nc.dram_tensor(
        "outPS", [128, npcols + NBLK * NCHUNK], f32, kind="ExternalOutput"
    )

    with tile.TileContext(nc) as tc:
        with (
            tc.tile_pool(name="sb", bufs=1) as sb,
            tc.tile_pool(name="ep", bufs=2) as ep,
            tc.tile_pool(name="pp", bufs=2, space="PSUM") as pp,
        ):
            # ft piece 0 first so the first matmuls (lhsT cols 0:128, rhs cols
            # 0:512) can start as early as possible; the rest stream behind.
            ft = sb.tile([128, N], f16)
            pieces = [(0, 512), (512, 2048), (2048, 4096), (4096, 6144),
                      (6144, 8192)]
            nc.sync.dma_start(ft[:, 0:512], ft_d[:, 0:512])
            ident = sb.tile([128, 128], f16)
            nc.scalar.dma_start(ident[:], ident_d[:])
            negd = sb.tile([128, 128], f16)
            nc.scalar.dma_start(negd[:], negd_d[:])
            for lo, hi in pieces[1:]:
                nc.sync.dma_start(ft[:, lo:hi], ft_d[:, lo:hi])

            biast = sb.tile([128, 1], f32)
            nc.vector.memset(biast[:], bias)

            plan = _piece_plan()
            npcols = sum(len(p) for p in plan)
            acc = sb.tile([128, npcols + NBLK * NCHUNK], f32)
            PN = npcols  # first npcols cols: P pieces; last 32: S chunks

            pcol = 0
            for m in range(NBLK):
                E = ep.tile([128, N], f16)
                d0 = m * 128
                qd = d0 // 512
                for k in range(NCHUNK):
                    ps = pp.tile([128, CHUNK], f32, tag="ps")
                    for q in range(CHUNK // 512):
                        nc.tensor.matmul(
                            ps[:, q * 512:(q + 1) * 512],
                            lhsT=ft[:, m * 128:(m + 1) * 128],
                            rhs=ft[:, k * CHUNK + q * 512:k * CHUNK + (q + 1) * 512],
                            start=True,
                            stop=not (k == 0 and q == qd),
                        )
                        if k == 0 and q == qd:
                            # accumulate -60000 onto the diagonal 128 cols
                            nc.tensor.matmul(
                                ps[:, d0:d0 + 128],
                                lhsT=ident[:], rhs=negd[:],
                                start=False, stop=True,
                            )
                    # exp with fused row-sum (S partial, one col per (m,k))
                    sc = PN + m * NCHUNK + k
                    nc.scalar.activation(
                        E[:, k * CHUNK:(k + 1) * CHUNK], ps[:],
                        mybir.ActivationFunctionType.Exp,
                        scale=float(1.0 / TAU), bias=biast[:],
                        accum_out=acc[:, sc:sc + 1],
                    )
                    done = (k + 1) * CHUNK
                    for lo, hi in plan[m]:
                        if hi != done:
                            continue
                        # fused 1x DVE pass per piece:
                        #   E = (E >= beta') * E, accum -> P piece
                        nc.vector.scalar_tensor_tensor(
                            out=E[:, lo:hi], in0=E[:, lo:hi],
                            scalar=betap, in1=E[:, lo:hi],
                            op0=Alu.is_ge, op1=Alu.mult,
                            accum_out=acc[:, pcol:pcol + 1],
                        )
                        pcol += 1

            nc.sync.dma_start(out_d[:], acc[:])
    nc.compile()
    return nc


def _prep_inputs(features: np.ndarray, alpha) -> tuple[list[dict], float]:
    feats = np.ascontiguousarray(np.asarray(features, dtype=np.float32))
    assert feats.shape == (N, D), feats.shape
    a = float(np.asarray(alpha, dtype=np.float32))

    norms = np.sqrt((feats.astype(np.float64) ** 2).sum(axis=1, keepdims=True))
    norms = np.maximum(norms, 1e-12)
    fn = (feats / norms).astype(np.float32)
    fT = np.ascontiguousarray(fn.T.astype(np.float16))  # [128, 8192] fp16

    ident = np.eye(128, dtype=np.float16)
    negd = (np.eye(128) * DIAG_NEG).astype(np.float16)

    in_maps = []
    for c in range(NCORES):
        ftc = np.ascontiguousarray(np.roll(fT, -c * R, axis=1))
        in_maps.append({"ft": ftc, "ident": ident, "negd": negd})
    return in_maps, a


def _assemble(results, alpha: float) -> np.float32:
    c = _shift_center(alpha)
    factor = np.exp(np.float64(c) / TAU)
    P = np.empty(N, np.float64)
    S = np.empty(N, np.float64)
    plan = _piece_plan()
    PN = sum(len(p) for p in plan)
    for ci in range(NCORES):
        o = np.asarray(results[ci]["outPS"], dtype=np.float64)
        Pm = np.empty((128, NBLK))
        pcol = 0
        for m in range(NBLK):
            npc = len(plan[m])
            Pm[:, m] = o[:, pcol:pcol + npc].sum(axis=1)
            pcol += npc
        Sm = o[:, PN:].reshape(128, NBLK, NCHUNK).sum(axis=2)    # [128, NBLK]
        P[ci * R:(ci + 1) * R] = Pm.T.reshape(R)
        S[ci * R:(ci + 1) * R] = Sm.T.reshape(R)
    P *= factor
    S *= factor
    num = P + EPS
    den = num + (S - P) + EPS
    loss = np.mean(np.log(den) - np.log(num))
    return np.float32(loss)


def kernel(features, alpha):
    from concourse.bass_utils import run_bass_kernel_spmd

    global LAST_RESULT
    in_maps, a = _prep_inputs(features, alpha)
    if a not in _CACHE:
        _CACHE[a] = _build(a)
    nc = _CACHE[a]
    res = run_bass_kernel_spmd(
        nc, in_maps, list(range(NCORES)), trace=PROFILE
    )
    LAST_RESULT = res
    return _assemble(res.results, a)

